# revision 15
# baseline (speedup 1.0000x reference)
"""DynamicEdgeConv GNN (3x EdgeConv + encoder) on 8 TRN2 NeuronCores.

Data-parallel over graphs: 16 graphs of 2048 nodes; 2 graphs per core.
Per graph-conv: hT [H=128, N=2048] kept feature-major in SBUF.
  scores(i,j) = h_i . h_j - 0.5*||h_j||^2   (argtop8 == kNN by distance)
  top-8 via DVE max / max_index, gather rows via indirect DMA from a DRAM
  copy of h, message MLP via PE with the [xi, xj-xi] concat rewritten as
  xi@(A-B) + xj@B, max-aggregate over k via DVE tensor_reduce on a strided
  view. Bias of the encoder is folded as a K=5 matmul; -0.5||h_j||^2 is
  folded as a K=1 ones matmul into the scores PSUM group.
"""

import numpy as np
from contextlib import ExitStack

import concourse.bass as bass
import concourse.mybir as mybir
from concourse import tile
from concourse.masks import make_identity

B_ALL = 16      # graphs total
N = 2048        # nodes per graph
KNN = 8
H = 128
F_IN = 4
CORES = 8
GPC = B_ALL // CORES          # graphs per core
NPC = GPC * N                 # nodes per core
NCH = N // 128                # 16 chunks of 128 nodes per graph
NB = N // 512                 # 4 blocks of 512 nodes per graph

FP = mybir.dt.float32
U32 = mybir.dt.uint32
AF = mybir.ActivationFunctionType
ALU = mybir.AluOpType
AX = mybir.AxisListType

CONV_TAGS = ["1", "2", "5"]

WEIGHT_SPECS = {
    "W_enc": (F_IN, H), "b_enc": (1, H),
    "W1a": (2 * H, H), "b1a": (H, 1), "W1b": (H, H), "b1b": (H, 1),
    "W2a": (2 * H, H), "b2a": (H, 1), "W2b": (H, H), "b2b": (H, 1),
    "W5a": (2 * H, H), "b5a": (H, 1), "W5b": (H, 1), "b5b": (1, 1),
}


def emit(tc, x, out_d, W):
    nc = tc.nc
    with ExitStack() as ctx:
        consts = ctx.enter_context(tc.tile_pool(name="consts", bufs=1))
        hpool = ctx.enter_context(tc.tile_pool(name="hpool", bufs=3))
        work = ctx.enter_context(tc.tile_pool(name="work", bufs=2))
        strips = ctx.enter_context(tc.tile_pool(name="strips", bufs=1))
        psum = ctx.enter_context(tc.tile_pool(name="psum", bufs=2, space="PSUM"))
        hdram = ctx.enter_context(tc.tile_pool(name="hdram", bufs=1, space="DRAM"))

        ident = consts.tile([128, 128], FP, tag="ident", name="ident")
        make_identity(nc, ident)
        ones_col = consts.tile([128, 1], FP, tag="ones_col", name="ones_col")
        nc.vector.memset(ones_col, 1.0)
        ones_row = consts.tile([1, 128], FP, tag="ones_row", name="ones_row")
        nc.vector.memset(ones_row, 1.0)
        ones_512 = consts.tile([1, 512], FP, tag="ones_512", name="ones_512")
        nc.vector.memset(ones_512, 1.0)

        w_enc_sb = consts.tile([F_IN, H], FP, tag="w_enc_sb", name="w_enc_sb")
        nc.sync.dma_start(w_enc_sb, W["W_enc"])
        b_enc_sb = consts.tile([1, H], FP, tag="b_enc_sb", name="b_enc_sb")
        nc.sync.dma_start(b_enc_sb, W["b_enc"])

        convW = []
        for t in CONV_TAGS:
            AB = consts.tile([H, 2 * H], FP, tag=f"AB{t}", name=f"AB{t}")
            nc.sync.dma_start(AB.rearrange("h (a j) -> h a j", a=2),
                              W[f"W{t}a"].rearrange("(a h) j -> h a j", a=2))
            Bm = AB[:, H:2 * H]
            AmB = consts.tile([H, H], FP, tag=f"AmB{t}", name=f"AmB{t}")
            nc.vector.tensor_sub(AmB, AB[:, 0:H], Bm)
            ba = consts.tile([H, 1], FP, tag=f"ba{t}", name=f"ba{t}")
            nc.sync.dma_start(ba, W[f"b{t}a"])
            if t != "5":
                Wb = consts.tile([H, H], FP, tag=f"Wb{t}", name=f"Wb{t}")
                bb = consts.tile([H, 1], FP, tag=f"bb{t}", name=f"bb{t}")
            else:
                Wb = consts.tile([H, 1], FP, tag=f"Wb{t}", name=f"Wb{t}")
                bb = consts.tile([1, 1], FP, tag=f"bb{t}", name=f"bb{t}")
            nc.sync.dma_start(Wb, W[f"W{t}b"])
            nc.sync.dma_start(bb, W[f"b{t}b"])
            convW.append((AmB, Bm, ba, Wb, bb))

        h_tab = [[hdram.tile([N, H], FP, tag=f"ht_{g}_{c}", name=f"ht_{g}_{c}")
                  for c in range(3)] for g in range(GPC)]

        # x transposed into SBUF (feature-major)
        xT = consts.tile([F_IN, NPC], FP, tag="xT", name="xT")
        nc.sync.dma_start(xT, x.rearrange("n f -> f n"))

        def store_htab(g, c, hT_src):
            dst = h_tab[g][c].rearrange("(cb q p) f -> cb p q f", q=4, p=128)
            for cb in range(4):
                pst = psum.tile([128, 512], FP, tag="t", name="pst_st")
                for q in range(4):
                    col = (cb * 4 + q) * 128
                    nc.tensor.transpose(pst[:, q * 128:(q + 1) * 128],
                                        hT_src[:, col:col + 128], ident)
                hsb = work.tile([128, 512], FP, tag="hst", name="hsb")
                nc.scalar.activation(hsb, pst, AF.Copy)
                nc.sync.dma_start(dst[cb], hsb.rearrange("p (q f) -> p q f", q=4))

        def edge_conv(g, conv, hT_in):
            AmB, Bm, ba, Wb, bb = convW[conv]

            h2 = work.tile([H, N], FP, tag="h2", name="h2")
            nc.scalar.activation(h2, hT_in, AF.Square)
            neghalf = strips.tile([1, N], FP, tag="nh", name="neghalf")
            for jb in range(NB):
                ps = psum.tile([128, 512], FP, tag="s", name="ps_sq")
                nc.tensor.matmul(ps[0:1, :], ones_col, h2[:, jb * 512:(jb + 1) * 512],
                                 start=True, stop=True)
                nc.scalar.activation(neghalf[:, jb * 512:(jb + 1) * 512], ps[0:1, :],
                                     AF.Copy, scale=-0.5)

            # unique idx tile per (g, conv): avoids WAR waits from the 8
            # SWDGE gather queues landing on max_index (1-wait-slot limit)
            idx = consts.tile([128, NCH * KNN], U32, tag=f"idx_{g}_{conv}",
                              name=f"idx_{g}_{conv}")
            for ci in range(NCH):
                sc = work.tile([128, N], FP, tag="sc", name="sc")
                for jb in range(NB):
                    ps = psum.tile([128, 512], FP, tag="s", name="ps_sc")
                    nc.tensor.matmul(ps, hT_in[:, ci * 128:(ci + 1) * 128],
                                     hT_in[:, jb * 512:(jb + 1) * 512],
                                     start=True, stop=False)
                    nc.tensor.matmul(ps, ones_row, neghalf[:, jb * 512:(jb + 1) * 512],
                                     start=False, stop=True)
                    nc.scalar.activation(sc[:, jb * 512:(jb + 1) * 512], ps, AF.Copy)
                vals = work.tile([128, 8], FP, tag="vals", name="vals")
                nc.vector.max(vals, sc)
                nc.vector.max_index(idx[:, ci * KNN:(ci + 1) * KNN], vals, sc)

            if conv < 2:
                hT_out = hpool.tile([H, N], FP, tag="hT", name="hT_out")
            else:
                out_row = strips.tile([1, N], FP, tag="outrow", name="out_row")
            for ib in range(NB):
                if conv < 2:
                    msgs = work.tile([128, KNN * 512], FP, tag="msgs", name="msgs")
                else:
                    m5 = strips.tile([1, KNN * 512], FP, tag="m5", name="m5")
                for k in range(KNN):
                    pst = psum.tile([128, 512], FP, tag="t", name="pst_xj")
                    for q in range(4):
                        ci = ib * 4 + q
                        xj = work.tile([128, H], FP, tag="xj", name="xj")
                        nc.gpsimd.indirect_dma_start(
                            out=xj, out_offset=None,
                            in_=h_tab[g][conv],
                            in_offset=bass.IndirectOffsetOnAxis(
                                ap=idx[:, ci * KNN + k: ci * KNN + k + 1], axis=0),
                        )
                        nc.tensor.transpose(pst[:, q * 128:(q + 1) * 128], xj, ident)
                    xjT = work.tile([H, 512], FP, tag="xjT", name="xjT")
                    nc.scalar.activation(xjT, pst, AF.Copy)
                    ps1 = psum.tile([128, 512], FP, tag="m1", name="ps1")
                    nc.tensor.matmul(ps1, Bm, xjT, start=True, stop=False)
                    nc.tensor.matmul(ps1, AmB, hT_in[:, ib * 512:(ib + 1) * 512],
                                     start=False, stop=True)
                    h1 = work.tile([H, 512], FP, tag="h1", name="h1")
                    nc.scalar.activation(h1, ps1, AF.Relu, bias=ba)
                    if conv < 2:
                        ps2 = psum.tile([128, 512], FP, tag="m2", name="ps2")
                        nc.tensor.matmul(ps2, Wb, h1, start=True, stop=True)
                        nc.scalar.activation(msgs[:, k * 512:(k + 1) * 512], ps2,
                                             AF.Relu, bias=bb)
                    else:
                        ps2 = psum.tile([1, 512], FP, tag="m2", name="ps2s")
                        nc.tensor.matmul(ps2, Wb, h1, start=True, stop=True)
                        nc.scalar.activation(m5[:, k * 512:(k + 1) * 512], ps2,
                                             AF.Relu, bias=bb)
                if conv < 2:
                    nc.vector.tensor_reduce(
                        out=hT_out[:, ib * 512:(ib + 1) * 512],
                        in_=msgs.rearrange("p (k i) -> p i k", k=KNN),
                        axis=AX.X, op=ALU.max)
                else:
                    nc.vector.tensor_reduce(
                        out=out_row[:, ib * 512:(ib + 1) * 512],
                        in_=m5.rearrange("p (k i) -> p i k", k=KNN),
                        axis=AX.X, op=ALU.max)
            if conv < 2:
                store_htab(g, conv + 1, hT_out)
                return hT_out
            # sigmoid after max (monotone), then store this graph's 2048 outputs
            sg_row = strips.tile([1, N], FP, tag="sgrow", name="sg_row")
            nc.scalar.activation(sg_row, out_row, AF.Sigmoid)
            dst = out_d.rearrange("(g n) one -> g one n", g=GPC)
            nc.sync.dma_start(dst[g], sg_row)
            return None

        for g in range(GPC):
            hT_cur = hpool.tile([H, N], FP, tag="hT", name="hT_enc")
            for jb in range(NB):
                ps = psum.tile([128, 512], FP, tag="s", name="ps_enc")
                nc.tensor.matmul(ps, w_enc_sb,
                                 xT[:, g * N + jb * 512: g * N + (jb + 1) * 512],
                                 start=True, stop=False)
                nc.tensor.matmul(ps, b_enc_sb, ones_512, start=False, stop=True)
                nc.scalar.activation(hT_cur[:, jb * 512:(jb + 1) * 512], ps, AF.Copy)
            store_htab(g, 0, hT_cur)
            for conv in range(3):
                hT_cur = edge_conv(g, conv, hT_cur)


def build():
    nc = bass.Bass("TRN2", target_bir_lowering=False, debug=False)
    x_d = nc.dram_tensor("x", [NPC, F_IN], FP, kind="ExternalInput")
    w_aps = {}
    for name, shape in WEIGHT_SPECS.items():
        w_aps[name] = nc.dram_tensor(name, list(shape), FP, kind="ExternalInput")[:]
    out_d = nc.dram_tensor("out", [NPC, 1], FP, kind="ExternalOutput")
    with tile.TileContext(nc) as tc:
        emit(tc, x_d[:], out_d[:], w_aps)
    # walrus CoreV3 codegen allows at most 1 sync wait per instruction;
    # split multi-wait instructions via event semaphores (Bacc passes)
    import bass_rust
    bass_rust.move_matmul_waits_to_ldweights(nc.m)
    bass_rust.generate_event_semaphores(nc)
    return nc


def make_in_maps(inputs):
    def f32(a):
        return np.ascontiguousarray(np.asarray(a), dtype=np.float32)
    w = {name: f32(inputs[name]).reshape(shape)
         for name, shape in WEIGHT_SPECS.items()}
    x_full = f32(inputs["x"])
    in_maps = []
    for c in range(CORES):
        m = dict(w)
        m["x"] = np.ascontiguousarray(x_full[c * NPC:(c + 1) * NPC])
        in_maps.append(m)
    return in_maps


def run(inputs, trace=False):
    from concourse.bass_utils import run_bass_kernel_spmd
    nc = build()
    in_maps = make_in_maps(inputs)
    res = run_bass_kernel_spmd(nc, in_maps, list(range(CORES)), trace=trace)
    out = np.concatenate(
        [np.asarray(res.results[c]["out"], dtype=np.float32) for c in range(CORES)],
        axis=0)
    return out, res


def kernel(**inputs):
    out, _ = run(inputs, trace=False)
    return out
